# revision 1
# baseline (speedup 1.0000x reference)
"""Single-head causal self-attention on 8 TRN2 NeuronCores.

Problem (hardcoded): x [8, 2048, 1024] f32, Wq/Wk/Wv [1024, 1024] f32.
  Q = x@Wq; K = x@Wk; V = x@Wv
  A = (Q K^T) / sqrt(1024), causal; P = softmax(A); out = P V   -> [8, 2048, 1024] f32

Sharding: batch-parallel — core b computes batch element b, no collectives.
Host-side marshaling per core: x[b] is transposed and cast to fp16 (x^T
[1024, 2048]), weights cast to fp16. This is input layout/dtype prep only
(no arithmetic); all FLOPs run on device.

Per-core algorithm (fp16 matmul inputs, fp32 PSUM accumulation):
  - Q^T, K^T in [dk, s] layout: matmul(lhsT=W chunk [d,dk], rhs=x^T [d,s]).
    V in natural [s, dv]: matmul(lhsT=x^T chunk [d,s], rhs=Wv [d,dv]).
  - Attention in transposed-score space, streamed over q-chunks of 256:
    S^T[k,q] = matmul(lhsT=K^T [dk,k], rhs=Q^T [dk,q]) accumulated over dk,
    E = exp(S^T/32) on ACT (no max subtraction: causal |scores| <= ~6),
    causal mask post-exp via affine_select on diagonal blocks,
    row sums r via ones-matmul with E^T as stationary (out [q,1]),
    O' accumulated in PSUM via matmul(lhsT=E^T block, rhs=V block),
    O = O' * (1/r) per-partition, DMA out as f32.
"""
import numpy as np

import concourse.bacc as bacc
import concourse.bass as bass
import concourse.mybir as mybir
import concourse.tile as tile

F32 = mybir.dt.float32
F16 = mybir.dt.float16

B = 8
S = 2048
D = 1024
P = 128
ND = D // P          # 8 d-tiles (and dk-tiles)
NS = S // P          # 16 s-tiles (k-tiles / q-tiles)
QC = 256             # q-chunk for attention
NQC = S // QC        # 8 chunks
INV_SCALE = 1.0 / 32.0   # 1/sqrt(d_model)


def build():
    nc = bacc.Bacc(None, target_bir_lowering=False)

    xt_d = nc.dram_tensor("xt", [D, S], F16, kind="ExternalInput")
    # WqT and WkT stacked: one DMA per paired chunk (per-dma fixed cost
    # is ~2us on the HWDGE ring, so fewer+combined transfers start sooner)
    wqkt_d = nc.dram_tensor("WqkT", [2, D, D], F16, kind="ExternalInput")
    wv_d = nc.dram_tensor("Wv", [D, D], F16, kind="ExternalInput")
    out_d = nc.dram_tensor("out", [S, D], F32, kind="ExternalOutput")

    with tile.TileContext(nc) as tc:
        with (
            tc.tile_pool(name="consts", bufs=1) as consts,
            tc.tile_pool(name="big", bufs=1) as big,
        ):
            ones = consts.tile([P, 1], F16)
            nc.gpsimd.memset(ones[:], 1.0)

            xt16 = big.tile([P, ND, S], F16)   # x^T: [d%128, d//128, s]
            tt16 = big.tile([P, ND, S], F16)   # T^T: [e%128, e//128, q], T = x G
            v16 = big.tile([P, NS, D], F16)    # V:   [s%128, s//128, dv]
            wv16 = big.tile([P, ND, D], F16)
            g16 = big.tile([P, ND, D], F16)    # G = Wq Wk^T: [d%128, d//128, e]
            # W^T pairs in 4 column-chunk tiles: dependency tracking is
            # tile-granular, so separate tiles let the first G groups start
            # as soon as the first chunk lands; wq/wk ride one DMA each.
            wqk_t = [big.tile([P, 2, ND, 256], F16, name=f"wqk{c}")
                     for c in range(4)]

            # ---- input DMAs, one HWDGE ring, FIFO in consumption order ----
            xt_src = xt_d[:, :].rearrange("(a p) s -> p a s", p=P)
            wqk_src = wqkt_d[:, :, :].rearrange("w (a p) n -> p w a n", p=P)
            wv_src = wv_d[:, :].rearrange("(a p) n -> p a n", p=P)
            for c in range(4):
                sl = slice(256 * c, 256 * (c + 1))
                nc.sync.dma_start(wqk_t[c][:], wqk_src[:, :, :, sl])
            nc.sync.dma_start(xt16[:], xt_src)
            nc.sync.dma_start(wv16[:], wv_src)

            with (
                tc.tile_pool(name="projp", bufs=3, space="PSUM") as projp,
                tc.tile_pool(name="warmp", bufs=1, space="PSUM") as warmp,
            ):
                ncopy = 0

                def psum_out(dst_ap, ps, k):
                    if k % 2 == 0:
                        nc.vector.tensor_copy(dst_ap, ps[:])
                    else:
                        nc.scalar.copy(dst_ap, ps[:])

                # Small PE warmup sized to end as the first W^T chunks land:
                # gets the HAM clock gate to 2.4 GHz before the G matmuls.
                dum = consts.tile([P, 512], F16)
                nc.gpsimd.memset(dum[:], 0.0)
                wtile = warmp.tile([P, 512], F32)
                for _ in range(12):
                    nc.tensor.matmul(wtile[:], dum[:, 0:P], dum[:],
                                     start=True, stop=True)

                # G = Wq Wk^T: out[d 128 (tile m), e 256 (chunk c)], groups
                # ordered so group level L only needs the first L+1 W^T
                # chunk tiles.
                gpairs = sorted(
                    ((m, c) for m in range(ND) for c in range(D // 256)),
                    key=lambda mc: (max(mc[0] // 2, mc[1]), mc[1], mc[0]))
                for m, c in gpairs:
                    ps = projp.tile([P, 256], F32, name="gps")
                    for a in range(ND):
                        nc.tensor.matmul(
                            ps[:],
                            wqk_t[m // 2][:, 0, a,
                                          P * (m % 2):P * (m % 2 + 1)],
                            wqk_t[c][:, 1, a, :],
                            start=(a == 0), stop=(a == ND - 1))
                    psum_out(g16[:, m, 256 * c:256 * (c + 1)], ps, ncopy)
                    ncopy += 1
                # T^T = G^T-contracted with x^T: out[e 128 (m), q 512 (c)]
                for c in range(S // 512):
                    for m in range(ND):
                        ps = projp.tile([P, 512], F32)
                        for a in range(ND):
                            nc.tensor.matmul(
                                ps[:],
                                g16[:, a, P * m:P * (m + 1)],
                                xt16[:, a, 512 * c:512 * (c + 1)],
                                start=(a == 0), stop=(a == ND - 1))
                        psum_out(tt16[:, m, 512 * c:512 * (c + 1)], ps, ncopy)
                        ncopy += 1
                # V: out[s 128 (i), dv 512 (h)]
                for i in range(NS):
                    for h in range(D // 512):
                        ps = projp.tile([P, 512], F32)
                        for a in range(ND):
                            nc.tensor.matmul(
                                ps[:],
                                xt16[:, a, P * i:P * (i + 1)],
                                wv16[:, a, 512 * h:512 * (h + 1)],
                                start=(a == 0), stop=(a == ND - 1))
                        psum_out(v16[:, i, 512 * h:512 * (h + 1)], ps, ncopy)
                        ncopy += 1

            # ---- attention over q-chunks of 256 (2 q-tiles: u=0,1) ----
            # Software-pipelined: each pass (j, t) computes S^T + exp; its
            # PV/r matmuls are deferred PIPE passes so later passes' S^T
            # work hides the exp->mask latency, across chunk boundaries.
            PIPE = 2
            with (
                tc.tile_pool(name="stp", bufs=2, space="PSUM") as stp,
                tc.tile_pool(name="op", bufs=1, space="PSUM") as op_,
                tc.tile_pool(name="rp", bufs=1, space="PSUM") as rp,
                tc.tile_pool(name="ep", bufs=PIPE + 3) as ep,
                tc.tile_pool(name="osbp", bufs=2) as osbp,
                tc.tile_pool(name="rrp", bufs=2) as rrp,
            ):
                o_ps = {}
                r_ps = {}

                def emit_st_exp(j, t):
                    tl = t - 2 * j       # diagonal-block local index
                    # last diagonal tile (tl==1) only sees q-sub u=1
                    qlo = QC * j + (P if tl == 1 else 0)
                    qw = P if tl == 1 else QC
                    st = stp.tile([P, QC], F32, name="st")
                    for m in range(ND):
                        nc.tensor.matmul(
                            st[:, 0:qw],
                            xt16[:, m, P * t:P * (t + 1)],
                            tt16[:, m, qlo:qlo + qw],
                            start=(m == 0), stop=(m == ND - 1))
                    et = ep.tile([P, QC], F16, name="et")
                    nc.scalar.activation(
                        et[:, 0:qw], st[:, 0:qw],
                        mybir.ActivationFunctionType.Exp,
                        scale=INV_SCALE)
                    if tl >= 0:
                        # keep iff k <= q  <=>  y - x >= 0 in local coords
                        # (the tl==1 tile holds q-local 128..255 in cols
                        #  0..127, so the same predicate applies)
                        nc.gpsimd.affine_select(
                            out=et[:, 0:qw], in_=et[:, 0:qw],
                            compare_op=mybir.AluOpType.is_ge,
                            fill=0.0, base=0,
                            pattern=[[1, qw]], channel_multiplier=-1)
                    return et

                def emit_pv(j, t, et):
                    nkt = 2 * j + 2
                    tl = t - 2 * j
                    if t == 0:
                        o_ps[j] = [op_.tile([P, D], F32, name=f"o_ps{u}")
                                   for u in range(2)]
                        r_ps[j] = [rp.tile([P, 1], F32, name=f"r_ps{u}")
                                   for u in range(2)]
                    for u in range(2):
                        if u == 0 and tl == 1:
                            continue  # fully masked
                        col = 0 if (u == 0 or tl == 1) else P
                        lhsT = et[:, col:col + P]
                        last = (t == nkt - 2) if u == 0 else (t == nkt - 1)
                        # short N=1 r matmul first; the two N=512 PV streams
                        # then hide the next group's LDWEIGHTS
                        nc.tensor.matmul(
                            r_ps[j][u][:], lhsT, ones[:],
                            start=(t == 0), stop=last)
                        for h in range(D // 512):
                            nc.tensor.matmul(
                                o_ps[j][u][:, 512 * h:512 * (h + 1)],
                                lhsT,
                                v16[:, t, 512 * h:512 * (h + 1)],
                                start=(t == 0), stop=last)
                    if t == nkt - 1:
                        emit_norm(j)

                def emit_norm(j):
                    rrec = rrp.tile([P, 2], F32, name="rrec")
                    for u in range(2):
                        nc.vector.reciprocal(rrec[:, u:u + 1], r_ps[j][u][:])
                    for u in range(2):
                        osb = osbp.tile([P, D], F32, name="osb")
                        # split across DVE and ACT so the PSUM banks free
                        # ~2x sooner at chunk boundaries
                        qt = 2 * j + u
                        nc.vector.tensor_scalar_mul(
                            osb[:, 0:512], o_ps[j][u][:, 0:512],
                            rrec[:, u:u + 1])
                        nc.sync.dma_start(
                            out_d[P * qt:P * (qt + 1), 0:512], osb[:, 0:512])
                        nc.scalar.activation(
                            osb[:, 512:D], o_ps[j][u][:, 512:D],
                            mybir.ActivationFunctionType.Copy,
                            scale=rrec[:, u:u + 1])
                        nc.sync.dma_start(
                            out_d[P * qt:P * (qt + 1), 512:D], osb[:, 512:D])
                    del o_ps[j], r_ps[j]

                passes = [(j, t) for j in range(NQC) for t in range(2 * j + 2)]
                pending = []
                for (j, t) in passes:
                    et = emit_st_exp(j, t)
                    pending.append((j, t, et))
                    if len(pending) > PIPE:
                        emit_pv(*pending.pop(0))
                for args in pending:
                    emit_pv(*args)

    nc.finalize()
    return nc


_NC = None


def _get_nc():
    global _NC
    if _NC is None:
        _NC = build()
    return _NC


def prep_inputs(x, Wq, Wk, Wv):
    """Host-side marshaling: shard batch, transpose+cast x, cast weights."""
    WqkT16 = np.ascontiguousarray(
        np.stack([np.asarray(Wq).T, np.asarray(Wk).T]), dtype=np.float16)
    Wv16 = np.ascontiguousarray(Wv, dtype=np.float16)
    return [
        {"xt": np.ascontiguousarray(np.asarray(x[b]).T, dtype=np.float16),
         "WqkT": WqkT16, "Wv": Wv16}
        for b in range(B)
    ]


def run(x, Wq, Wk, Wv, **spmd_kwargs):
    from concourse.bass_utils import run_bass_kernel_spmd

    nc = _get_nc()
    in_maps = prep_inputs(x, Wq, Wk, Wv)
    res = run_bass_kernel_spmd(nc, in_maps, core_ids=list(range(B)),
                               **spmd_kwargs)
    out = np.stack([res.results[b]["out"] for b in range(B)], axis=0)
    return out, res


def kernel(x, Wq, Wk, Wv):
    out, _ = run(x, Wq, Wk, Wv)
    return out



# revision 5
# speedup vs baseline: 1.4908x; 1.4908x over previous
"""Single-head causal self-attention on 8 TRN2 NeuronCores.

Problem (hardcoded): x [8, 2048, 1024] f32, Wq/Wk/Wv [1024, 1024] f32.
  Q = x@Wq; K = x@Wk; V = x@Wv
  A = (Q K^T) / sqrt(1024), causal; P = softmax(A); out = P V   -> [8, 2048, 1024] f32

Sharding: batch-parallel - core b computes batch element b, no collectives.

Mixed-precision fp8/fp16 design (rel-err budget 2e-2, fp16 baseline 5.9e-4):
  - The first q-chunk (rows 0..255) has a concentrated softmax (row q
    averages over <=q+1 keys), which amplifies input quantization noise
    into the output; it runs fully in fp16 (projections + attention).
  - Rows 256.. have diffuse softmax (sqrt(sum P^2) <= ~0.2), so fp8e4
    noise (~4-7% per score) attenuates to <~1e-2 of output absmax.
    Everything on that path runs in fp8e4 with DoubleRow perf mode
    (2 contraction elements / PE cycle): T = x(WqWk^T), V = xWv,
    S^T = x^T-contract-T^T, and P@V.
  - G = Wq Wk^T stays fp16 (it feeds the fp16 j=0 path); the fp8 copy is
    scaled x16 so its sigma=1/32 entries clear the e4m3 subnormal floor,
    with the 1/16 folded into the exp scale. Wv8 is likewise host-scaled
    x16, cancelled by a 16-valued ones vector in the row-sum matmul.
  - Causal masking: a [128,128] identity matmul adds a -1600/-25600 bias
    tile into the diagonal score PSUM; exp(bias*scale) flushes masked
    entries to exact 0 in fp8/fp16. No gpsimd in the attention loop.
  - exp uses bias -3 (E = e^(s-3)) so E stays below e4m3's 240->Inf
    cliff even for 7-sigma scores; the common factor cancels in r.
"""
import numpy as np

import concourse.bacc as bacc
import concourse.bass as bass
import concourse.mybir as mybir
import concourse.tile as tile

F32 = mybir.dt.float32
F16 = mybir.dt.float16
F8 = mybir.dt.float8e4
DR = mybir.MatmulPerfMode.DoubleRow

B = 8
S = 2048
D = 1024
P = 128
ND = D // P          # 8 d-tiles (contraction tiles)
NS = S // P          # 16 s-tiles
QC = 256             # q-chunk for attention
NQC = S // QC        # 8 chunks
INV16 = 1.0 / 32.0        # exp scale for fp16 scores (1/sqrt(d_model))
INV8 = 1.0 / (32.0 * 16.0)  # fp8 scores carry the x16 of g8
EXPB = -3.0               # exp bias: E = e^(s-3), cancels in normalize
VS = 16.0                 # host scale on Wv8, cancelled via ones8=16


def build():
    nc = bacc.Bacc(None, target_bir_lowering=False)

    xt8_d = nc.dram_tensor("xt8", [D, S], F8, kind="ExternalInput")
    xt16_d = nc.dram_tensor("xt16", [D, QC], F16, kind="ExternalInput")
    wqkt_d = nc.dram_tensor("WqkT", [2, D, D], F16, kind="ExternalInput")
    wv16_d = nc.dram_tensor("Wv16", [D, D], F16, kind="ExternalInput")
    wv8_d = nc.dram_tensor("Wv8", [D, D], F8, kind="ExternalInput")
    id_d = nc.dram_tensor("id16", [P, P], F16, kind="ExternalInput")
    # mask bias rows: [tl, variant(fp16=-1600 / fp8=-25600)]
    mb_d = nc.dram_tensor("maskb", [P, 4, QC], F16, kind="ExternalInput")
    ones8_d = nc.dram_tensor("ones8", [P, 2, 1], F8, kind="ExternalInput")
    out_d = nc.dram_tensor("out", [S, D], F32, kind="ExternalOutput")

    with tile.TileContext(nc) as tc:
        with (
            tc.tile_pool(name="consts", bufs=1) as consts,
            tc.tile_pool(name="big", bufs=1) as big,
        ):
            ones16 = consts.tile([P, 1], F16)
            nc.gpsimd.memset(ones16[:], 1.0)
            expb = consts.tile([P, 1], F32)
            nc.gpsimd.memset(expb[:], EXPB)

            xt16 = big.tile([P, ND, QC], F16)   # x^T cols 0:256, fp16
            x8 = big.tile([P, ND, S], F8)       # x^T full, fp8
            tt16 = big.tile([P, ND, QC], F16)   # T^T q-cols 0:256, fp16
            t8 = big.tile([P, ND, S], F8)       # 16*T^T, q-cols 256:, fp8
            v16 = big.tile([P, 2, D], F16)      # V rows 0:256, fp16
            v8 = big.tile([P, NS, D], F8)       # 16*V, fp8
            g16 = big.tile([P, ND, D], F16)     # G = Wq Wk^T
            g8 = big.tile([P, ND, D], F8)       # 16*G
            wv16s = big.tile([P, ND, D], F16)
            wv8s = big.tile([P, ND, D], F8)
            id16 = consts.tile([P, P], F16)
            maskb = consts.tile([P, 4, QC], F16)
            ones8 = consts.tile([P, 2, 1], F8)
            wqk_t = [big.tile([P, 2, ND, 256], F16, name=f"wqk{c}")
                     for c in range(4)]

            # ---- input DMAs, one ring, FIFO in consumption order ----
            wqk_src = wqkt_d[:, :, :].rearrange("w (a p) n -> p w a n", p=P)
            for c in range(4):
                sl = slice(256 * c, 256 * (c + 1))
                nc.sync.dma_start(wqk_t[c][:], wqk_src[:, :, :, sl])
            nc.sync.dma_start(id16[:], id_d[:, :])
            nc.sync.dma_start(maskb[:], mb_d[:, :, :])
            nc.sync.dma_start(ones8[:], ones8_d[:, :, :])
            nc.sync.dma_start(
                xt16[:], xt16_d[:, :].rearrange("(a p) s -> p a s", p=P))
            nc.sync.dma_start(
                x8[:], xt8_d[:, :].rearrange("(a p) s -> p a s", p=P))
            nc.sync.dma_start(
                wv16s[:], wv16_d[:, :].rearrange("(a p) n -> p a n", p=P))
            nc.sync.dma_start(
                wv8s[:], wv8_d[:, :].rearrange("(a p) n -> p a n", p=P))

            with (
                tc.tile_pool(name="projp", bufs=3, space="PSUM") as projp,
                tc.tile_pool(name="warmp", bufs=1, space="PSUM") as warmp,
            ):
                ncopy = 0

                def psum_out(dst_ap, ps, k):
                    if k % 2 == 0:
                        nc.vector.tensor_copy(dst_ap, ps[:])
                    else:
                        nc.scalar.copy(dst_ap, ps[:])

                # PE warmup sized to end as the first W^T chunks land.
                dum = consts.tile([P, 512], F16)
                nc.gpsimd.memset(dum[:], 0.0)
                wtile = warmp.tile([P, 512], F32)
                for _ in range(12):
                    nc.tensor.matmul(wtile[:], dum[:, 0:P], dum[:],
                                     start=True, stop=True)

                # G = Wq Wk^T (fp16): out[d 128 (m), e 256 (c)], group
                # order so level L only needs the first L+1 W^T chunks.
                gpairs = sorted(
                    ((m, c) for m in range(ND) for c in range(D // 256)),
                    key=lambda mc: (max(mc[0] // 2, mc[1]), mc[1], mc[0]))
                for m, c in gpairs:
                    ps = projp.tile([P, 256], F32, name="gps")
                    for a in range(ND):
                        nc.tensor.matmul(
                            ps[:],
                            wqk_t[m // 2][:, 0, a,
                                          P * (m % 2):P * (m % 2 + 1)],
                            wqk_t[c][:, 1, a, :],
                            start=(a == 0), stop=(a == ND - 1))
                    # two copies: fp16 master (DVE) + x16 fp8 (ACT)
                    sl = slice(256 * c, 256 * (c + 1))
                    nc.vector.tensor_copy(g16[:, m, sl], ps[:])
                    nc.scalar.activation(
                        g8[:, m, sl], ps[:],
                        mybir.ActivationFunctionType.Copy, scale=VS)

                # T^T q-cols 0:256 (fp16): out[e 128 (m), q 256]
                for m in range(ND):
                    ps = projp.tile([P, QC], F32)
                    for a in range(ND):
                        nc.tensor.matmul(
                            ps[:],
                            g16[:, a, P * m:P * (m + 1)],
                            xt16[:, a, :],
                            start=(a == 0), stop=(a == ND - 1))
                    psum_out(tt16[:, m, :], ps, ncopy)
                    ncopy += 1
                # V rows 0:256 (fp16): out[s 128 (i), dv 512 (h)]
                for i in range(2):
                    for h in range(D // 512):
                        ps = projp.tile([P, 512], F32)
                        for a in range(ND):
                            nc.tensor.matmul(
                                ps[:],
                                xt16[:, a, P * i:P * (i + 1)],
                                wv16s[:, a, 512 * h:512 * (h + 1)],
                                start=(a == 0), stop=(a == ND - 1))
                        psum_out(v16[:, i, 512 * h:512 * (h + 1)], ps, ncopy)
                        ncopy += 1

                # 16*T^T q-cols 256:2048 (fp8 DoubleRow over d)
                qchunks = [(256, 512), (768, 512), (1280, 512), (1792, 256)]
                for qlo, qw in qchunks:
                    for m in range(ND):
                        ps = projp.tile([P, 512], F32)
                        for a in range(ND // 2):
                            nc.tensor.matmul(
                                ps[:, 0:qw],
                                g8[:, 2 * a:2 * a + 2, P * m:P * (m + 1)],
                                x8[:, 2 * a:2 * a + 2, qlo:qlo + qw],
                                start=(a == 0), stop=(a == ND // 2 - 1),
                                perf_mode=DR)
                        psum_out(t8[:, m, qlo:qlo + qw], ps[:, 0:qw], ncopy)
                        ncopy += 1
                # 16*V (fp8 DoubleRow): out[s 128 (i), dv 512 (h)]
                for i in range(NS):
                    for h in range(D // 512):
                        ps = projp.tile([P, 512], F32)
                        for a in range(ND // 2):
                            nc.tensor.matmul(
                                ps[:],
                                x8[:, 2 * a:2 * a + 2, P * i:P * (i + 1)],
                                wv8s[:, 2 * a:2 * a + 2,
                                     512 * h:512 * (h + 1)],
                                start=(a == 0), stop=(a == ND // 2 - 1),
                                perf_mode=DR)
                        psum_out(v8[:, i, 512 * h:512 * (h + 1)], ps, ncopy)
                        ncopy += 1

            # ---- attention over q-chunks of 256, k-tile PAIRS of 256 ----
            # Pass (j, c) computes S^T + exp for k-tiles 2c, 2c+1 into a
            # paired E tile [P, 2, QC]; PV/r run as deferred PIPE passes
            # (DoubleRow over the k-pair for j>=1, fp16 per-tile for j=0).
            PIPE = 2
            with (
                tc.tile_pool(name="stp", bufs=2, space="PSUM") as stp,
                tc.tile_pool(name="op", bufs=1, space="PSUM") as op_,
                tc.tile_pool(name="rp", bufs=1, space="PSUM") as rp,
                tc.tile_pool(name="ep", bufs=PIPE + 3) as ep,
                tc.tile_pool(name="osbp", bufs=2) as osbp,
                tc.tile_pool(name="rrp", bufs=2) as rrp,
            ):
                o_ps = {}
                r_ps = {}

                def emit_pair_scores(j, c):
                    fp8 = j > 0
                    et = ep.tile([P, 2, QC], F8 if fp8 else F16, name="et")
                    for tpar in range(2):
                        t = 2 * c + tpar
                        diag = (t - 2 * j) >= 0  # diagonal-block k-tile
                        st = stp.tile([P, QC], F32, name="st")
                        if fp8:
                            for a in range(ND // 2):
                                nc.tensor.matmul(
                                    st[:],
                                    x8[:, 2 * a:2 * a + 2, P * t:P * (t + 1)],
                                    t8[:, 2 * a:2 * a + 2,
                                       QC * j:QC * (j + 1)],
                                    start=(a == 0),
                                    stop=(a == ND // 2 - 1) and not diag,
                                    perf_mode=DR)
                        else:
                            for a in range(ND):
                                nc.tensor.matmul(
                                    st[:],
                                    xt16[:, a, P * t:P * (t + 1)],
                                    tt16[:, a, :],
                                    start=(a == 0),
                                    stop=(a == ND - 1) and not diag)
                        if diag:
                            # add -1600/-25600 masked-position bias rows
                            mvar = 2 * (t - 2 * j) + (1 if fp8 else 0)
                            nc.tensor.matmul(
                                st[:], id16[:], maskb[:, mvar, :],
                                start=False, stop=True)
                        nc.scalar.activation(
                            et[:, tpar, :], st[:],
                            mybir.ActivationFunctionType.Exp,
                            scale=INV8 if fp8 else INV16, bias=expb[:])
                    return et

                def emit_pv(j, c, et):
                    fp8 = j > 0
                    npair = j + 1
                    if c == 0:
                        o_ps[j] = [op_.tile([P, D], F32, name=f"o_ps{u}")
                                   for u in range(2)]
                        r_ps[j] = [rp.tile([P, 1], F32, name=f"r_ps{u}")
                                   for u in range(2)]
                    for u in range(2):
                        last = (c == npair - 1)
                        if fp8:
                            lhsT = et[:, :, P * u:P * (u + 1)]
                            nc.tensor.matmul(
                                r_ps[j][u][:], lhsT, ones8[:],
                                start=(c == 0), stop=last, perf_mode=DR)
                            for h in range(D // 512):
                                nc.tensor.matmul(
                                    o_ps[j][u][:, 512 * h:512 * (h + 1)],
                                    lhsT,
                                    v8[:, 2 * c:2 * c + 2,
                                       512 * h:512 * (h + 1)],
                                    start=(c == 0), stop=last, perf_mode=DR)
                        else:
                            # j=0, single pair: tile 1 is all-zero for u=0
                            tp = [0] if u == 0 else [0, 1]
                            for ti, tpar in enumerate(tp):
                                lhsT = et[:, tpar, P * u:P * (u + 1)]
                                st_, sp_ = (ti == 0), (ti == len(tp) - 1)
                                nc.tensor.matmul(
                                    r_ps[j][u][:], lhsT, ones16[:],
                                    start=st_, stop=sp_)
                                for h in range(D // 512):
                                    nc.tensor.matmul(
                                        o_ps[j][u][:, 512 * h:512 * (h + 1)],
                                        lhsT,
                                        v16[:, tpar, 512 * h:512 * (h + 1)],
                                        start=st_, stop=sp_)
                    if c == npair - 1:
                        emit_norm(j)

                def emit_norm(j):
                    rrec = rrp.tile([P, 2], F32, name="rrec")
                    for u in range(2):
                        nc.vector.reciprocal(rrec[:, u:u + 1], r_ps[j][u][:])
                    for u in range(2):
                        osb = osbp.tile([P, D], F32, name="osb")
                        # split across DVE and ACT so the PSUM banks free
                        # ~2x sooner at chunk boundaries
                        qt = 2 * j + u
                        nc.vector.tensor_scalar_mul(
                            osb[:, 0:512], o_ps[j][u][:, 0:512],
                            rrec[:, u:u + 1])
                        nc.sync.dma_start(
                            out_d[P * qt:P * (qt + 1), 0:512], osb[:, 0:512])
                        nc.scalar.activation(
                            osb[:, 512:D], o_ps[j][u][:, 512:D],
                            mybir.ActivationFunctionType.Copy,
                            scale=rrec[:, u:u + 1])
                        nc.sync.dma_start(
                            out_d[P * qt:P * (qt + 1), 512:D], osb[:, 512:D])
                    del o_ps[j], r_ps[j]

                passes = [(j, c) for j in range(NQC) for c in range(j + 1)]
                pending = []
                for (j, c) in passes:
                    et = emit_pair_scores(j, c)
                    pending.append((j, c, et))
                    if len(pending) > PIPE:
                        emit_pv(*pending.pop(0))
                for args in pending:
                    emit_pv(*args)

    nc.finalize()
    return nc


_NC = None


def _get_nc():
    global _NC
    if _NC is None:
        _NC = build()
    return _NC


def prep_inputs(x, Wq, Wk, Wv):
    """Host-side marshaling: shard batch, transpose + cast, constants."""
    import ml_dtypes
    F8NP = ml_dtypes.float8_e4m3

    WqkT16 = np.ascontiguousarray(
        np.stack([np.asarray(Wq).T, np.asarray(Wk).T]), dtype=np.float16)
    Wv16 = np.ascontiguousarray(Wv, dtype=np.float16)
    Wv8 = np.ascontiguousarray(np.asarray(Wv) * VS).astype(F8NP)
    id16 = np.eye(P, dtype=np.float16)
    r = np.arange(P)[:, None]
    col = np.arange(QC)[None, :]
    maskb = np.zeros((P, 4, QC), dtype=np.float16)
    maskb[:, 0, :] = np.where(col >= r, 0.0, -1600.0)
    maskb[:, 1, :] = np.where(col >= r, 0.0, -25600.0)
    maskb[:, 2, :] = np.where(col >= r + P, 0.0, -1600.0)
    maskb[:, 3, :] = np.where(col >= r + P, 0.0, -25600.0)
    ones8 = np.full((P, 2, 1), VS).astype(F8NP)

    out = []
    for b in range(B):
        xt = np.ascontiguousarray(np.asarray(x[b]).T)
        out.append({
            "xt8": xt.astype(F8NP),
            "xt16": np.ascontiguousarray(xt[:, 0:QC]).astype(np.float16),
            "WqkT": WqkT16, "Wv16": Wv16, "Wv8": Wv8,
            "id16": id16, "maskb": maskb, "ones8": ones8,
        })
    return out


def run(x, Wq, Wk, Wv, **spmd_kwargs):
    from concourse.bass_utils import run_bass_kernel_spmd

    nc = _get_nc()
    in_maps = prep_inputs(x, Wq, Wk, Wv)
    res = run_bass_kernel_spmd(nc, in_maps, core_ids=list(range(B)),
                               **spmd_kwargs)
    out = np.stack([res.results[b]["out"] for b in range(B)], axis=0)
    return out, res


def kernel(x, Wq, Wk, Wv):
    out, _ = run(x, Wq, Wk, Wv)
    return out
